# revision 51
# baseline (speedup 1.0000x reference)
"""TRN2 Bass kernel: fused multi-head attention (GPT-2 style, no causal mask).

Computes, for full inputs:
    qkv = X @ c_attn_w + c_attn_b ; q,k,v = split(qkv)
    per head: P = softmax(q k^T / sqrt(64)) ; a = P v
    out = merge_heads(a) @ c_proj_w + c_proj_b

Sharding: tensor-parallel over heads. 16 heads across 8 cores -> 2 heads/core.
Partials summed on the host (fp16 partials, fp32 host accumulate).

v5 (on the v2 software-pipelined loop): three structural changes keep the PE
dense to the end of the kernel (v2 spent its last 81us mostly at half clock,
HAM-throttled on a sparse dependency-serialized tail):

1. Decoupled streams.  QKV for batches 1-3 is one stream drained 4 MMs/step
   (finished by step 36 of 64), and the S^T+exp stream then runs up to 12
   steps AHEAD of the OT stream (pt pool deepened to 14), so groups 6-7 -
   which have no QKV filler - arrive with their exps already computed and
   run pure OT+cproj back-to-back on the PE.
2. Cheap, low-latency group boundaries.  The softmax denominator chain is
   bit-trick seed + one Newton step (3 DVE ops) -> -1/den bf16 -> K=1 PE
   broadcast matmul -> ACT copy -> DVE normalize (sign folded into the
   normalize).  ACT otherwise runs exps only; fills and most cproj
   evacuations ride DVE.  Group 6's OT h0 accumulator lives in the
   then-idle qkv PSUM bank so OT(g,0) never waits on the previous group's
   normalize.
3. Smoothed output.  cproj drains 1/step from group 2 (2/step in groups
   6-7) with its PSUM tile rotating through the ST pool, output DMAs
   trickle 4/step on the sync queue and alternate sync/scalar in the
   epilogue, which interleaves the last norm chain with the remaining
   cproj tiles.
"""

import os
from contextlib import ExitStack

import ml_dtypes
import numpy as np

import concourse.bass as bass
import concourse.mybir as mybir
from concourse import bacc, tile
from concourse.bass import broadcast_tensor_aps
from concourse.bass_utils import run_bass_kernel_spmd

F32 = mybir.dt.float32
BF16 = mybir.dt.bfloat16
F16 = mybir.dt.float16
I32 = mybir.dt.int32

B, S, NX = 4, 1024, 1024
T = B * S  # 4096 tokens
NCORES = 8
HD = 64  # head dim
V2S = 2 * (HD + 1)  # 130 columns per 128-token block in the V2 layout
EXP = mybir.ActivationFunctionType.Exp
RECIP_MAGIC = 0x7EF311C3  # fp32 bit-trick reciprocal seed, ~5% rel err

_nc_cache = None


def _ensure_ntff_hook():
    """The agent image's `antenv` lacks `axon_hooks`; synthesize it (see
    trn_agent_boot). Returns True if profiling is available."""
    import sys
    import types

    try:
        from antenv.axon_hooks import get_axon_ntff_profile_hook  # noqa: F401

        return True
    except ImportError:
        pass
    try:
        import antenv
        from trn_agent_boot.trn_boot import _ntff_profile_via_ctypes

        mod = types.ModuleType("antenv.axon_hooks")
        mod._hook = _ntff_profile_via_ctypes("/opt/axon/libaxon_pjrt.so")

        def set_axon_ntff_profile_hook(h):
            mod._hook = h

        def get_axon_ntff_profile_hook():
            return mod._hook

        mod.set_axon_ntff_profile_hook = set_axon_ntff_profile_hook
        mod.get_axon_ntff_profile_hook = get_axon_ntff_profile_hook
        sys.modules["antenv.axon_hooks"] = mod
        antenv.axon_hooks = mod
        return True
    except Exception as e:  # pragma: no cover - profiling is best-effort
        print(f"kernel.py: NTFF profile hook unavailable ({e}); running untraced")
        return False


def _emit(nc, tc, xtr, wq, wk, wv, wp, bq, bk, bv, identd, onecd, out):
    with ExitStack() as ctx:
        const = ctx.enter_context(tc.tile_pool(name="const", bufs=1))
        wq_sb = const.tile([128, 1024], BF16, tag="wq")
        wk_sb = const.tile([128, 1024], BF16, tag="wk")
        wv_sb = const.tile([128, 1024], BF16, tag="wv")
        wp_sb = const.tile([128, 1024], BF16, tag="wp")
        bq_sb = const.tile([128, 1], F32, tag="bq")
        bk_sb = const.tile([128, 1], F32, tag="bk")
        bv_sb = const.tile([128, 1], F32, tag="bv")
        ident = const.tile([128, 128], BF16, tag="ident")
        onec = const.tile([128, 64], BF16, tag="onec")
        # full X^T resident in SBUF: xt_all[p, qc, k, n] = X[qc*512+n, k*128+p]
        xt_all = const.tile([128, 8, 8, 512], BF16, tag="xt_all")
        qt = const.tile([128, T], BF16, tag="qt")
        kt = const.tile([128, T], BF16, tag="kt")
        vt = const.tile([128, T], BF16, tag="vt")
        v2 = const.tile([128, (T // 128) * V2S], BF16, tag="v2")
        atall = const.tile([128, T], BF16, tag="atall")
        at1 = const.tile([64, T], BF16, tag="at1")

        # xt stream rides the scalar queue (its own DMA ring) so the first
        # fills aren't queued behind the weight transfers on sync
        nc.sync.dma_start(ident[:], identd)
        nc.scalar.dma_start(xt_all[:, 0, :, :], xtr[:, 0, :, :])
        nc.sync.dma_start(wq_sb[:], wq)
        nc.sync.dma_start(wk_sb[:], wk)
        nc.scalar.dma_start(xt_all[:, 1, :, :], xtr[:, 1, :, :])
        nc.sync.dma_start(wv_sb[:], wv)
        nc.sync.dma_start(bq_sb[:], bq)
        nc.sync.dma_start(bk_sb[:], bk)
        nc.sync.dma_start(bv_sb[:], bv)
        nc.sync.dma_start(wp_sb[:], wp)
        nc.sync.dma_start(onec[:], onecd)
        for qc in range(2, 8):
            nc.scalar.dma_start(xt_all[:, qc, :, :], xtr[:, qc, :, :])
        # ones columns of V2: per token block, col 64 (head 0) and col 129 (head 1)
        v2_ones = v2[:].rearrange("p (t h e) -> p t h e", h=2, e=HD + 1)[
            :, :, :, HD : HD + 1
        ]
        nc.vector.tensor_copy(
            v2_ones, onec[:].rearrange("p (t h e) -> p t h e", h=2, e=1)
        )

        # PSUM (8 banks): stP 2x[128,1024]F32 = 4, qkvP 1x[128,512]F32 = 1,
        # msP 1x[128,512] = 1, otP 2x[65,512]F32 = 2.
        stP = ctx.enter_context(tc.tile_pool(name="stP", bufs=2, space="PSUM"))
        qkvP = ctx.enter_context(tc.tile_pool(name="qkvP", bufs=1, space="PSUM"))
        msP = ctx.enter_context(tc.tile_pool(name="msP", bufs=1, space="PSUM"))
        otP = ctx.enter_context(tc.tile_pool(name="otP", bufs=2, space="PSUM"))
        ptp = ctx.enter_context(tc.tile_pool(name="ptp", bufs=14))
        rcp = ctx.enter_context(tc.tile_pool(name="rcp", bufs=6))
        bcp = ctx.enter_context(tc.tile_pool(name="bcp", bufs=4))
        obp = ctx.enter_context(tc.tile_pool(name="obp", bufs=24))

        # ---- PE warmup: ~3.8us of cheap N=64 matmuls while the weight/xt
        # DMAs land, so the HAM clock gate reaches K=8/8 before real work.
        warm = stP.tile([128, 1024], F32, tag="st", name="warm")
        for _ in range(72):
            nc.tensor.matmul(
                warm[:, 0:64], ident[:], ident[:, 0:64], start=True, stop=True
            )

        # ---------- QKV machinery (for one batch = 48 matmuls, 6 fills) ----
        w_for = {0: wq_sb, 1: wk_sb, 2: wv_sb}
        b_for = {0: bq_sb, 1: bk_sb, 2: bv_sb}
        d_for = {0: qt, 1: kt, 2: vt}
        qkv_state = {}  # live psum tile for the running fill

        # q/k fills first so the first S^T pair of the batch's first group
        # never waits behind the v-fill evacuations on the DVE queue
        FILL_MAP = ((0, 0), (1, 0), (0, 1), (1, 1), (2, 0), (2, 1))

        def emit_qkv_mm(bn, m):
            """m-th of the 48 QKV matmuls for batch bn; returns pending
            (transpose work) when a v-fill completes."""
            fill, chunk = divmod(m, 8)
            tgt, qch = FILL_MAP[fill]
            qc = 2 * bn + qch
            if chunk == 0:
                qkv_state["ps"] = qkvP.tile([128, 512], F32, tag="fill", name="ps")
            ps = qkv_state["ps"]
            nc.tensor.matmul(
                ps[:],
                w_for[tgt][:, chunk * 128 : (chunk + 1) * 128],
                xt_all[:, qc, chunk, :],
                start=(chunk == 0),
                stop=(chunk == 7),
            )
            if chunk == 7:
                # PSUM->SBUF cast + per-partition bias, on DVE (ACT is kept
                # free for the exps)
                dst = d_for[tgt]
                nc.vector.tensor_scalar_add(
                    dst[:, qc * 512 : (qc + 1) * 512], ps[:], b_for[tgt][:, 0:1]
                )
                if tgt == 2:
                    return qc  # v-fill complete -> transposes pending
            return None

        def emit_v_transposes(qc):
            """V^T -> token-major V2 for one q-chunk (4 PE transposes + copy)."""
            tp = msP.tile([128, 512], BF16, tag="msc", name="tp")
            for t4 in range(4):
                nc.tensor.transpose(
                    tp[:, t4 * 128 : (t4 + 1) * 128],
                    vt[:, qc * 512 + t4 * 128 : qc * 512 + (t4 + 1) * 128],
                    ident[:],
                )
            src = tp[:].rearrange("p (t h e) -> p t h e", h=2, e=HD)
            dst = v2[:].rearrange("p (t h e) -> p t h e", h=2, e=HD + 1)[
                :, qc * 4 : (qc + 1) * 4, :, 0:HD
            ]
            nc.vector.tensor_copy(dst, src)

        # ---------- prologue: batch-0 q/k fills only, dense on the PE ----
        # (the v fills join the main stream, so the first in-loop fill never
        # WAR-stalls on the last prologue evacuation)
        pend_tp = []
        for m in range(32):
            emit_qkv_mm(0, m)

        # ---------- softmax tail helpers ----------
        rrows = {}  # (g, h) -> sbuf fp32 row tile holding 1/den at row 64
        bcs = {}  # (g, h) -> [64, 512] fp32 bc tile

        def emit_recip(g, h):
            """3-op DVE chain on the [1,512] denominator row (row 64 of the
            OT psum tile): bit-trick seed + one Newton step -> 1/den fp32."""
            den = ots[(g, h)][64:65, :]
            sd = rcp.tile([65, 512], I32, tag="r", name="sd")
            nc.vector.tensor_scalar(
                sd[64:65, :],
                den.bitcast(I32),
                -1,
                RECIP_MAGIC,
                mybir.AluOpType.mult,
                mybir.AluOpType.add,
            )
            y0 = sd[64:65, :].bitcast(F32)
            t1 = rcp.tile([65, 512], F32, tag="r", name="t1")
            nc.vector.tensor_mul(t1[64:65, :], den, y0)
            m1 = rcp.tile([65, 512], BF16, tag="r", name="m1")  # -1/den
            nc.vector.scalar_tensor_tensor(
                m1[64:65, :],
                t1[64:65, :],
                2.0,
                y0,
                mybir.AluOpType.subtract,
                mybir.AluOpType.mult,
            )
            rrows[(g, h)] = m1

        def emit_bcast(g, h):
            """Broadcast -1/den to 64 partitions: PE matmul with a K=1 ones
            row (no DMA queues on this chain - they jam at group boundaries),
            then ACT copy to SBUF."""
            m1 = rrows.pop((g, h))
            bcb = msP.tile([64, 512], F32, tag="msc", name="bcb")
            nc.tensor.matmul(
                bcb[:], onec[64:65, 0:64], m1[64:65, :], start=True, stop=True
            )
            bc = bcp.tile([64, 512], F32, tag="bc", name="bc")
            nc.scalar.copy(bc[:], bcb[:])
            bcs[(g, h)] = bc

        def emit_norm(g, h):
            """atall[h] = ot * (1/den), written bf16.  bc holds -1/den (the
            Newton chain's natural sign), so fold a -1 into the multiply."""
            b_, q2_ = divmod(g, 2)
            q0 = b_ * 1024 + q2_ * 512
            at = atall if h == 0 else at1
            bc = bcs.pop((g, h))
            nc.vector.scalar_tensor_tensor(
                at[0:64, q0 : q0 + 512],
                ots[(g, h)][0:64, :],
                -1.0,
                bc[0:64, :],
                mybir.AluOpType.mult,
                mybir.AluOpType.mult,
            )
            if h == 1:
                nc.scalar.dma_start(
                    atall[64:128, q0 : q0 + 512], at1[0:64, q0 : q0 + 512]
                )
                ots.pop((g, 0))
                ots.pop((g, 1))

        # ---------- cproj helpers ----------
        cproj_fifo = []  # (cb, oc, t2) ready to compute
        cproj_done = []  # (cb, oc, t2, ob_tile) copied, awaiting DMA
        cproj_eng = [0]

        def emit_cproj(cb, oc, t2, pool, tag, period=4):
            """period=4 -> 3:1 DVE:ACT copies (main loop); period=2 ->
            alternate (late groups/epilogue, where DVE saturates first)."""
            op_t = pool.tile([128, 512], F32, tag=tag, name="op")
            nc.tensor.matmul(
                op_t[:],
                wp_sb[:, oc * 128 : (oc + 1) * 128],
                atall[:, cb * 1024 + t2 * 512 : cb * 1024 + (t2 + 1) * 512],
                start=True,
                stop=True,
            )
            ob = obp.tile([128, 512], F16, tag="ob", name="ob")
            cproj_eng[0] = (cproj_eng[0] + 1) % period
            if cproj_eng[0] == 0:
                nc.scalar.copy(ob[:], op_t[:])
            else:
                nc.vector.tensor_copy(ob[:], op_t[:])
            cproj_done.append((cb, oc, t2, ob))

        out_eng = [0]

        def emit_out_dmas(limit, queues=(nc.sync,)):
            n = 0
            while cproj_done and n < limit:
                cb, oc, t2, ob = cproj_done.pop(0)
                out_eng[0] = (out_eng[0] + 1) % len(queues)
                queues[out_eng[0]].dma_start(
                    out[
                        oc * 128 : (oc + 1) * 128,
                        cb * 1024 + t2 * 512 : cb * 1024 + (t2 + 1) * 512,
                    ],
                    ob[:],
                )
                n += 1

        # ---------- main pipelined loop over the 8 attention groups -------
        CPROJ_QUOTA = (0, 0, 1, 1, 2, 2, 2, 2)
        pts = {}  # (g, kb) -> pt tile [128, 1024] (h0 cols 0-511, h1 512-1023)
        ots = {}  # (g, h) -> ot psum tile

        def v2_col(b_, kb, h):
            return ((b_ * 8 + kb) * 2 + h) * (HD + 1)

        def emit_ot_pair(g, kb):
            # group 6's h0 tile lives in qkvP (idle once batch-3 fills end),
            # so the g5->g6 and g6->g7 norm chains each gate only one otP
            # slot and OT(g,0) stops stalling on the previous group's norm.
            b_ = g // 2
            for h in (0, 1):
                if kb == 0:
                    pool, tag = (qkvP, "fill") if (g == 6 and h == 0) else (otP, "ot")
                    ots[(g, h)] = pool.tile([65, 512], F32, tag=tag, name="ot")
                c = v2_col(b_, kb, h)
                nc.tensor.matmul(
                    ots[(g, h)][:],
                    v2[:, c : c + HD + 1],
                    pts[(g, kb)][:, h * 512 : (h + 1) * 512],
                    start=(kb == 0),
                    stop=(kb == 7),
                )
            pts.pop((g, kb))

        def emit_st_pair(g, s):
            b_, q2_ = divmod(g, 2)
            q0 = b_ * 1024 + q2_ * 512
            k0 = b_ * 1024 + s * 128
            st = stP.tile([128, 1024], F32, tag="st", name="st")
            nc.tensor.matmul(
                st[:, 0:512],
                kt[0:64, k0 : k0 + 128],
                qt[0:64, q0 : q0 + 512],
                start=True,
                stop=True,
            )
            nc.tensor.matmul(
                st[:, 512:1024],
                kt[64:128, k0 : k0 + 128],
                qt[64:128, q0 : q0 + 512],
                start=True,
                stop=True,
            )
            pt = ptp.tile([128, 1024], BF16, tag="pt", name="pt")
            nc.scalar.activation(pt[:], st[:], EXP, scale=0.125)
            pts[(g, s)] = pt

        # QKV for batches 1-3 is a single stream drained 4 MMs/step (done by
        # step 36); the S^T+exp stream then runs AHEAD of the OT stream (up
        # to 2/step once the QKV stream is dry), so by groups 6-7 every exp
        # has already been computed and the tail is pure OT+cproj on the PE.
        qkv_stream = [(0, m) for m in range(32, 48)]
        qkv_stream += [(bn, m) for bn in (1, 2, 3) for m in range(48)]
        # batch 3's v fills are deferred into the exp-limited run-ahead phase
        # (steps 40+), where the PE would otherwise idle ~1us/step and trip
        # the HAM clock gate; they still finish well before group 6's OT
        late_stream = [(3, m) for m in range(32, 48)]
        qkv_stream = [u for u in qkv_stream if u not in late_stream]
        st_stream = [divmod(i, 8) for i in range(64)]
        sp = [0]

        def drain_qkv(n, gstep=0, late_ok=False):
            for _ in range(n):
                src_q = (
                    qkv_stream
                    if qkv_stream
                    else (late_stream if late_ok and gstep >= 40 else None)
                )
                if src_q:
                    r = emit_qkv_mm(*src_q.pop(0))
                    if r is not None:
                        pend_tp.append(r)

        def emit_sts(gstep):
            # mandatory: the current group's S^T is never later than its own
            # step; once the QKV stream is dry, run up to 2/step ahead
            # (capped so the pt pool never backs the ACT queue up on
            # not-yet-ready inputs)
            n = 1 if qkv_stream else 2
            for _ in range(n):
                if sp[0] < 64 and sp[0] <= gstep + 12:
                    emit_st_pair(*st_stream[sp[0]])
                    sp[0] += 1

        OUTQ2 = (nc.sync, nc.scalar)

        for g in range(8):
            b_, q2_ = divmod(g, 2)

            for s in range(8):
                gstep = g * 8 + s
                # --- softmax tail of group g-1 FIRST: its ACT copy and
                # DVE norm enter the queues ahead of the run-ahead exps ---
                if g >= 1:
                    if s == 0:
                        emit_bcast(g - 1, 0)
                    elif s == 1:
                        emit_norm(g - 1, 0)
                        emit_bcast(g - 1, 1)
                    elif s == 2:
                        emit_norm(g - 1, 1)
                    elif s == 3:
                        pc_, pt2_ = divmod(g - 1, 2)
                        cproj_fifo.extend((pc_, oc, pt2_) for oc in range(8))
                # --- OT pair: lag 4, but lag 2 in the last two groups
                # (their exps are pre-computed by the S^T run-ahead), which
                # pulls the final norm chains ~2 steps earlier ---
                lag = 2 if g >= 6 else 4
                if s >= lag:
                    emit_ot_pair(g, s - lag)
                drain_qkv(2, gstep, late_ok=True)
                # --- S^T stream (current group's step, or run-ahead) ---
                emit_sts(gstep)
                drain_qkv(2, gstep, late_ok=True)
                # pending V transposes (1 slot per step keeps PE dense)
                if pend_tp and s % 2 == 1:
                    emit_v_transposes(pend_tp.pop(0))
                # --- cproj from the FIFO, psum rotating through stP ---
                for ci in range(CPROJ_QUOTA[g]):
                    if cproj_fifo:
                        emit_cproj(
                            *cproj_fifo.pop(0), stP, "st", period=2 if g >= 6 else 4
                        )
                # bunched OT tail at the end of the group; filler between the
                # last pairs so OT(g,7) never waits on exp(g,7)
                if s == 7:
                    # the OT pairs themselves cover exp(g,7)'s latency; the
                    # recips go out BEFORE any other DVE work so the norm
                    # chain isn't queued behind copies
                    if lag == 4:
                        emit_ot_pair(g, 4)
                        emit_ot_pair(g, 5)
                    if cproj_fifo and CPROJ_QUOTA[g]:
                        emit_cproj(*cproj_fifo.pop(0), stP, "st", period=1)
                    emit_ot_pair(g, 6)
                    if cproj_fifo and CPROJ_QUOTA[g]:
                        emit_cproj(*cproj_fifo.pop(0), stP, "st", period=1)
                    emit_ot_pair(g, 7)
                    emit_recip(g, 0)
                    emit_recip(g, 1)
                    if pend_tp:
                        emit_v_transposes(pend_tp.pop(0))
                # copied cproj tiles -> HBM, spread across mid-group steps so
                # the sync queue is clear for the bc DMAs at boundaries;
                # in the last two groups the exps are done, so the scalar
                # queue helps drain the output backlog
                if 1 <= s <= 6:
                    emit_out_dmas(4, OUTQ2 if g >= 6 else (nc.sync,))

            while pend_tp:
                emit_v_transposes(pend_tp.pop(0))

        # ---------- epilogue ----------
        # Tail of group 7 interleaved with the remaining cproj matmuls
        # (leftovers only need norm(6), which is done; then g7's own 8).
        emit_bcast(7, 0)
        ep_pools = ((stP, "st"), (msP, "msc"), (qkvP, "fill"))
        ep_i = [0]

        def ep_cproj(n):
            for _ in range(n):
                if cproj_fifo:
                    pool, tag = ep_pools[ep_i[0] % 3]
                    ep_i[0] += 1
                    emit_cproj(*cproj_fifo.pop(0), pool, tag, period=2)

        ep_cproj(2)
        emit_norm(7, 0)
        emit_bcast(7, 1)
        ep_cproj(2)
        emit_norm(7, 1)
        EPQ = (nc.sync, nc.scalar)
        ep_cproj(len(cproj_fifo))
        emit_out_dmas(4, EPQ)
        cproj_fifo.extend((3, oc, 1) for oc in range(8))
        while cproj_fifo:
            ep_cproj(2)
            emit_out_dmas(2, EPQ)
        emit_out_dmas(len(cproj_done), EPQ)


def _build_nc():
    nc = bacc.Bacc(
        "TRN2",
        target_bir_lowering=False,
        debug=False,
        enable_asserts=False,
        num_devices=NCORES,
    )
    xtr = nc.dram_tensor("xtr", [128, 8, 8, 512], BF16, kind="ExternalInput").ap()
    wq = nc.dram_tensor("wq", [128, 1024], BF16, kind="ExternalInput").ap()
    wk = nc.dram_tensor("wk", [128, 1024], BF16, kind="ExternalInput").ap()
    wv = nc.dram_tensor("wv", [128, 1024], BF16, kind="ExternalInput").ap()
    wp = nc.dram_tensor("wp", [128, 1024], BF16, kind="ExternalInput").ap()
    bq = nc.dram_tensor("bq", [128, 1], F32, kind="ExternalInput").ap()
    bk = nc.dram_tensor("bk", [128, 1], F32, kind="ExternalInput").ap()
    bv = nc.dram_tensor("bv", [128, 1], F32, kind="ExternalInput").ap()
    identd = nc.dram_tensor("ident", [128, 128], BF16, kind="ExternalInput").ap()
    onecd = nc.dram_tensor("onec", [128, 64], BF16, kind="ExternalInput").ap()
    out = nc.dram_tensor("out_t", [NX, T], F16, kind="ExternalOutput").ap()
    with tile.TileContext(nc) as tc:
        _emit(nc, tc, xtr, wq, wk, wv, wp, bq, bk, bv, identd, onecd, out)
    nc.compile()
    return nc


def _pack_w(wcols):
    # [1024, 128] -> [128, 8*128] bf16: sbuf[p, k*128 + j] = W[k*128 + p, j]
    w = np.ascontiguousarray(np.asarray(wcols, dtype=np.float32))
    return np.ascontiguousarray(
        w.reshape(8, 128, 128).transpose(1, 0, 2).reshape(128, 1024)
    ).astype(ml_dtypes.bfloat16)


def _pack_xtr(X):
    # X [T, NX] -> xtr[p, qc, k, n] = X[qc*512+n, k*128+p]
    xt = np.asarray(X, dtype=np.float32).T  # [NX, T]
    xtr = xt.reshape(8, 128, 8, 512).transpose(1, 2, 0, 3)
    return np.ascontiguousarray(xtr).astype(ml_dtypes.bfloat16)


def kernel(hidden_states, c_attn_w, c_attn_b, c_proj_w, c_proj_b):
    global _nc_cache
    hidden_states = np.asarray(hidden_states, dtype=np.float32)
    c_attn_w = np.asarray(c_attn_w, dtype=np.float32)
    c_attn_b = np.asarray(c_attn_b, dtype=np.float32)
    c_proj_w = np.asarray(c_proj_w, dtype=np.float32)
    c_proj_b = np.asarray(c_proj_b, dtype=np.float32)

    if _nc_cache is None:
        _nc_cache = _build_nc()
    nc = _nc_cache

    X = hidden_states.reshape(T, NX)
    xtr_np = _pack_xtr(X)

    in_maps = []
    for c in range(NCORES):
        cs = slice(c * 128, (c + 1) * 128)
        in_maps.append(
            {
                "xtr": xtr_np,
                "wq": _pack_w(c_attn_w[:, c * 128 : (c + 1) * 128]),
                "wk": _pack_w(c_attn_w[:, 1024 + c * 128 : 1024 + (c + 1) * 128]),
                "wv": _pack_w(c_attn_w[:, 2048 + c * 128 : 2048 + (c + 1) * 128]),
                "wp": np.ascontiguousarray(c_proj_w[cs, :]).astype(ml_dtypes.bfloat16),
                "bq": np.ascontiguousarray(c_attn_b[cs].reshape(128, 1)),
                "bk": np.ascontiguousarray(
                    c_attn_b[1024 + c * 128 : 1024 + (c + 1) * 128].reshape(128, 1)
                ),
                "bv": np.ascontiguousarray(
                    c_attn_b[2048 + c * 128 : 2048 + (c + 1) * 128].reshape(128, 1)
                ),
                "ident": np.eye(128, dtype=np.float32).astype(ml_dtypes.bfloat16),
                "onec": np.ones((128, 64), dtype=ml_dtypes.bfloat16),
            }
        )

    trace = bool(int(os.environ.get("KERNEL_PROFILE", "0")))
    if trace:
        trace = _ensure_ntff_hook()
    try:
        res = run_bass_kernel_spmd(
            nc, in_maps, core_ids=list(range(NCORES)), trace=trace
        )
    except Exception:
        if not trace:
            raise
        print("kernel.py: traced run failed; retrying untraced")
        res = run_bass_kernel_spmd(nc, in_maps, core_ids=list(range(NCORES)))

    total = np.zeros((NX, T), np.float32)
    for r in res.results:
        total += r["out_t"].astype(np.float32)
    out = total.T.reshape(B, S, NX) + c_proj_b[None, None, :]
    kernel.last_exec_time_ns = res.exec_time_ns
    return out.astype(np.float32)


# revision 52
# speedup vs baseline: 1.0253x; 1.0253x over previous
"""TRN2 Bass kernel: fused multi-head attention (GPT-2 style, no causal mask).

Computes, for full inputs:
    qkv = X @ c_attn_w + c_attn_b ; q,k,v = split(qkv)
    per head: P = softmax(q k^T / sqrt(64)) ; a = P v
    out = merge_heads(a) @ c_proj_w + c_proj_b

Sharding: tensor-parallel over heads. 16 heads across 8 cores -> 2 heads/core.
Partials summed on the host (fp16 partials, fp32 host accumulate).

v5 (on the v2 software-pipelined loop): three structural changes keep the PE
dense to the end of the kernel (v2 spent its last 81us mostly at half clock,
HAM-throttled on a sparse dependency-serialized tail):

1. Decoupled streams.  QKV for batches 1-3 is one stream drained 4 MMs/step
   (finished by step 36 of 64), and the S^T+exp stream then runs up to 12
   steps AHEAD of the OT stream (pt pool deepened to 14), so groups 6-7 -
   which have no QKV filler - arrive with their exps already computed and
   run pure OT+cproj back-to-back on the PE.
2. Cheap, low-latency group boundaries.  The softmax denominator chain is
   bit-trick seed + one Newton step (3 DVE ops) -> -1/den bf16 -> K=1 PE
   broadcast matmul -> ACT copy -> DVE normalize (sign folded into the
   normalize).  ACT otherwise runs exps only; fills and most cproj
   evacuations ride DVE.  Group 6's OT h0 accumulator lives in the
   then-idle qkv PSUM bank so OT(g,0) never waits on the previous group's
   normalize.
3. Smoothed output.  cproj drains 1/step from group 2 (2/step in groups
   6-7) with its PSUM tile rotating through the ST pool, output DMAs
   trickle 4/step on the sync queue and alternate sync/scalar in the
   epilogue, which interleaves the last norm chain with the remaining
   cproj tiles.
"""

import os
from contextlib import ExitStack

import ml_dtypes
import numpy as np

import concourse.bass as bass
import concourse.mybir as mybir
from concourse import bacc, tile
from concourse.bass import broadcast_tensor_aps
from concourse.bass_utils import run_bass_kernel_spmd

F32 = mybir.dt.float32
BF16 = mybir.dt.bfloat16
F16 = mybir.dt.float16
I32 = mybir.dt.int32

B, S, NX = 4, 1024, 1024
T = B * S  # 4096 tokens
NCORES = 8
HD = 64  # head dim
V2S = 2 * (HD + 1)  # 130 columns per 128-token block in the V2 layout
EXP = mybir.ActivationFunctionType.Exp
RECIP_MAGIC = 0x7EF311C3  # fp32 bit-trick reciprocal seed, ~5% rel err

_nc_cache = None


def _ensure_ntff_hook():
    """The agent image's `antenv` lacks `axon_hooks`; synthesize it (see
    trn_agent_boot). Returns True if profiling is available."""
    import sys
    import types

    try:
        from antenv.axon_hooks import get_axon_ntff_profile_hook  # noqa: F401

        return True
    except ImportError:
        pass
    try:
        import antenv
        from trn_agent_boot.trn_boot import _ntff_profile_via_ctypes

        mod = types.ModuleType("antenv.axon_hooks")
        mod._hook = _ntff_profile_via_ctypes("/opt/axon/libaxon_pjrt.so")

        def set_axon_ntff_profile_hook(h):
            mod._hook = h

        def get_axon_ntff_profile_hook():
            return mod._hook

        mod.set_axon_ntff_profile_hook = set_axon_ntff_profile_hook
        mod.get_axon_ntff_profile_hook = get_axon_ntff_profile_hook
        sys.modules["antenv.axon_hooks"] = mod
        antenv.axon_hooks = mod
        return True
    except Exception as e:  # pragma: no cover - profiling is best-effort
        print(f"kernel.py: NTFF profile hook unavailable ({e}); running untraced")
        return False


def _emit(nc, tc, xtr, wq, wk, wv, wp, bq, bk, bv, identd, onecd, out):
    with ExitStack() as ctx:
        const = ctx.enter_context(tc.tile_pool(name="const", bufs=1))
        wq_sb = const.tile([128, 1024], BF16, tag="wq")
        wk_sb = const.tile([128, 1024], BF16, tag="wk")
        wv_sb = const.tile([128, 1024], BF16, tag="wv")
        wp_sb = const.tile([128, 1024], BF16, tag="wp")
        bq_sb = const.tile([128, 1], F32, tag="bq")
        bk_sb = const.tile([128, 1], F32, tag="bk")
        bv_sb = const.tile([128, 1], F32, tag="bv")
        ident = const.tile([128, 128], BF16, tag="ident")
        onec = const.tile([128, 64], BF16, tag="onec")
        # full X^T resident in SBUF: xt_all[p, qc, k, n] = X[qc*512+n, k*128+p]
        xt_all = const.tile([128, 8, 8, 512], BF16, tag="xt_all")
        qt = const.tile([128, T], BF16, tag="qt")
        kt = const.tile([128, T], BF16, tag="kt")
        vt = const.tile([128, T], BF16, tag="vt")
        v2 = const.tile([128, (T // 128) * V2S], BF16, tag="v2")
        atall = const.tile([128, T], BF16, tag="atall")
        at1 = const.tile([64, T], BF16, tag="at1")

        # xt stream rides the scalar queue (its own DMA ring) so the first
        # fills aren't queued behind the weight transfers on sync
        nc.sync.dma_start(ident[:], identd)
        nc.scalar.dma_start(xt_all[:, 0, :, :], xtr[:, 0, :, :])
        nc.sync.dma_start(wq_sb[:], wq)
        nc.sync.dma_start(wk_sb[:], wk)
        nc.scalar.dma_start(xt_all[:, 1, :, :], xtr[:, 1, :, :])
        nc.sync.dma_start(wv_sb[:], wv)
        nc.sync.dma_start(bq_sb[:], bq)
        nc.sync.dma_start(bk_sb[:], bk)
        nc.sync.dma_start(bv_sb[:], bv)
        nc.sync.dma_start(wp_sb[:], wp)
        nc.sync.dma_start(onec[:], onecd)
        for qc in range(2, 8):
            nc.scalar.dma_start(xt_all[:, qc, :, :], xtr[:, qc, :, :])
        # ones columns of V2: per token block, col 64 (head 0) and col 129 (head 1)
        v2_ones = v2[:].rearrange("p (t h e) -> p t h e", h=2, e=HD + 1)[
            :, :, :, HD : HD + 1
        ]
        nc.vector.tensor_copy(
            v2_ones, onec[:].rearrange("p (t h e) -> p t h e", h=2, e=1)
        )

        # PSUM (8 banks): stP 2x[128,1024]F32 = 4, qkvP 1x[128,512]F32 = 1,
        # msP 1x[128,512] = 1, otP 2x[65,512]F32 = 2.
        stP = ctx.enter_context(tc.tile_pool(name="stP", bufs=2, space="PSUM"))
        qkvP = ctx.enter_context(tc.tile_pool(name="qkvP", bufs=1, space="PSUM"))
        msP = ctx.enter_context(tc.tile_pool(name="msP", bufs=1, space="PSUM"))
        otP = ctx.enter_context(tc.tile_pool(name="otP", bufs=2, space="PSUM"))
        ptp = ctx.enter_context(tc.tile_pool(name="ptp", bufs=14))
        rcp = ctx.enter_context(tc.tile_pool(name="rcp", bufs=6))
        bcp = ctx.enter_context(tc.tile_pool(name="bcp", bufs=4))
        obp = ctx.enter_context(tc.tile_pool(name="obp", bufs=24))

        # ---- PE warmup: ~3.8us of cheap N=64 matmuls while the weight/xt
        # DMAs land, so the HAM clock gate reaches K=8/8 before real work.
        warm = stP.tile([128, 1024], F32, tag="st", name="warm")
        for _ in range(72):
            nc.tensor.matmul(
                warm[:, 0:64], ident[:], ident[:, 0:64], start=True, stop=True
            )

        # ---------- QKV machinery (for one batch = 48 matmuls, 6 fills) ----
        w_for = {0: wq_sb, 1: wk_sb, 2: wv_sb}
        b_for = {0: bq_sb, 1: bk_sb, 2: bv_sb}
        d_for = {0: qt, 1: kt, 2: vt}
        qkv_state = {}  # live psum tile for the running fill

        # q/k fills first so the first S^T pair of the batch's first group
        # never waits behind the v-fill evacuations on the DVE queue
        FILL_MAP = ((0, 0), (1, 0), (0, 1), (1, 1), (2, 0), (2, 1))

        def emit_qkv_mm(bn, m):
            """m-th of the 48 QKV matmuls for batch bn; returns pending
            (transpose work) when a v-fill completes."""
            fill, chunk = divmod(m, 8)
            tgt, qch = FILL_MAP[fill]
            qc = 2 * bn + qch
            if chunk == 0:
                qkv_state["ps"] = qkvP.tile([128, 512], F32, tag="fill", name="ps")
            ps = qkv_state["ps"]
            nc.tensor.matmul(
                ps[:],
                w_for[tgt][:, chunk * 128 : (chunk + 1) * 128],
                xt_all[:, qc, chunk, :],
                start=(chunk == 0),
                stop=(chunk == 7),
            )
            if chunk == 7:
                # PSUM->SBUF cast + per-partition bias, on DVE (ACT is kept
                # free for the exps)
                dst = d_for[tgt]
                nc.vector.tensor_scalar_add(
                    dst[:, qc * 512 : (qc + 1) * 512], ps[:], b_for[tgt][:, 0:1]
                )
                if tgt == 2:
                    return qc  # v-fill complete -> transposes pending
            return None

        def emit_v_transposes(qc):
            """V^T -> token-major V2 for one q-chunk (4 PE transposes + copy)."""
            tp = msP.tile([128, 512], BF16, tag="msc", name="tp")
            for t4 in range(4):
                nc.tensor.transpose(
                    tp[:, t4 * 128 : (t4 + 1) * 128],
                    vt[:, qc * 512 + t4 * 128 : qc * 512 + (t4 + 1) * 128],
                    ident[:],
                )
            src = tp[:].rearrange("p (t h e) -> p t h e", h=2, e=HD)
            dst = v2[:].rearrange("p (t h e) -> p t h e", h=2, e=HD + 1)[
                :, qc * 4 : (qc + 1) * 4, :, 0:HD
            ]
            nc.vector.tensor_copy(dst, src)

        # ---------- prologue: batch-0 q/k fills only, dense on the PE ----
        # (the v fills join the main stream, so the first in-loop fill never
        # WAR-stalls on the last prologue evacuation)
        pend_tp = []
        for m in range(32):
            emit_qkv_mm(0, m)

        # ---------- softmax tail helpers ----------
        rrows = {}  # (g, h) -> sbuf fp32 row tile holding 1/den at row 64
        bcs = {}  # (g, h) -> [64, 512] fp32 bc tile

        def emit_recip(g, h):
            """3-op DVE chain on the [1,512] denominator row (row 64 of the
            OT psum tile): bit-trick seed + one Newton step -> 1/den fp32."""
            den = ots[(g, h)][64:65, :]
            sd = rcp.tile([65, 512], I32, tag="r", name="sd")
            nc.vector.tensor_scalar(
                sd[64:65, :],
                den.bitcast(I32),
                -1,
                RECIP_MAGIC,
                mybir.AluOpType.mult,
                mybir.AluOpType.add,
            )
            y0 = sd[64:65, :].bitcast(F32)
            t1 = rcp.tile([65, 512], F32, tag="r", name="t1")
            nc.vector.tensor_mul(t1[64:65, :], den, y0)
            m1 = rcp.tile([65, 512], BF16, tag="r", name="m1")  # -1/den
            nc.vector.scalar_tensor_tensor(
                m1[64:65, :],
                t1[64:65, :],
                2.0,
                y0,
                mybir.AluOpType.subtract,
                mybir.AluOpType.mult,
            )
            rrows[(g, h)] = m1

        def emit_bcast(g, h):
            """Broadcast -1/den to 64 partitions: PE matmul with a K=1 ones
            row (no DMA queues on this chain - they jam at group boundaries),
            then ACT copy to SBUF."""
            m1 = rrows.pop((g, h))
            bcb = msP.tile([64, 512], F32, tag="msc", name="bcb")
            nc.tensor.matmul(
                bcb[:], onec[64:65, 0:64], m1[64:65, :], start=True, stop=True
            )
            bc = bcp.tile([64, 512], F32, tag="bc", name="bc")
            nc.scalar.copy(bc[:], bcb[:])
            bcs[(g, h)] = bc

        def emit_norm(g, h):
            """atall[h] = ot * (1/den), written bf16.  bc holds -1/den (the
            Newton chain's natural sign), so fold a -1 into the multiply."""
            b_, q2_ = divmod(g, 2)
            q0 = b_ * 1024 + q2_ * 512
            at = atall if h == 0 else at1
            bc = bcs.pop((g, h))
            nc.vector.scalar_tensor_tensor(
                at[0:64, q0 : q0 + 512],
                ots[(g, h)][0:64, :],
                -1.0,
                bc[0:64, :],
                mybir.AluOpType.mult,
                mybir.AluOpType.mult,
            )
            if h == 1:
                nc.scalar.dma_start(
                    atall[64:128, q0 : q0 + 512], at1[0:64, q0 : q0 + 512]
                )
                ots.pop((g, 0))
                ots.pop((g, 1))

        # ---------- cproj helpers ----------
        cproj_fifo = []  # (cb, oc, t2) ready to compute
        cproj_done = []  # (cb, oc, t2, ob_tile) copied, awaiting DMA
        cproj_eng = [0]

        def emit_cproj(cb, oc, t2, pool, tag, period=4):
            """period=4 -> 3:1 DVE:ACT copies (main loop); period=2 ->
            alternate (late groups/epilogue, where DVE saturates first)."""
            op_t = pool.tile([128, 512], F32, tag=tag, name="op")
            nc.tensor.matmul(
                op_t[:],
                wp_sb[:, oc * 128 : (oc + 1) * 128],
                atall[:, cb * 1024 + t2 * 512 : cb * 1024 + (t2 + 1) * 512],
                start=True,
                stop=True,
            )
            ob = obp.tile([128, 512], F16, tag="ob", name="ob")
            cproj_eng[0] = (cproj_eng[0] + 1) % period
            if cproj_eng[0] == 0:
                nc.scalar.copy(ob[:], op_t[:])
            else:
                nc.vector.tensor_copy(ob[:], op_t[:])
            cproj_done.append((cb, oc, t2, ob))

        out_eng = [0]

        def emit_out_dmas(limit, queues=(nc.sync,)):
            n = 0
            while cproj_done and n < limit:
                cb, oc, t2, ob = cproj_done.pop(0)
                out_eng[0] = (out_eng[0] + 1) % len(queues)
                queues[out_eng[0]].dma_start(
                    out[
                        oc * 128 : (oc + 1) * 128,
                        cb * 1024 + t2 * 512 : cb * 1024 + (t2 + 1) * 512,
                    ],
                    ob[:],
                )
                n += 1

        # ---------- main pipelined loop over the 8 attention groups -------
        CPROJ_QUOTA = (0, 0, 1, 1, 2, 2, 2, 2)
        pts = {}  # (g, kb) -> pt tile [128, 1024] (h0 cols 0-511, h1 512-1023)
        ots = {}  # (g, h) -> ot psum tile

        def v2_col(b_, kb, h):
            return ((b_ * 8 + kb) * 2 + h) * (HD + 1)

        def emit_ot_pair(g, kb):
            # group 6's h0 tile lives in qkvP (idle once batch-3 fills end),
            # so the g5->g6 and g6->g7 norm chains each gate only one otP
            # slot and OT(g,0) stops stalling on the previous group's norm.
            b_ = g // 2
            for h in (0, 1):
                if kb == 0:
                    pool, tag = (qkvP, "fill") if (g == 6 and h == 0) else (otP, "ot")
                    ots[(g, h)] = pool.tile([65, 512], F32, tag=tag, name="ot")
                c = v2_col(b_, kb, h)
                nc.tensor.matmul(
                    ots[(g, h)][:],
                    v2[:, c : c + HD + 1],
                    pts[(g, kb)][:, h * 512 : (h + 1) * 512],
                    start=(kb == 0),
                    stop=(kb == 7),
                )
            pts.pop((g, kb))

        def emit_st_pair(g, s):
            b_, q2_ = divmod(g, 2)
            q0 = b_ * 1024 + q2_ * 512
            k0 = b_ * 1024 + s * 128
            st = stP.tile([128, 1024], F32, tag="st", name="st")
            nc.tensor.matmul(
                st[:, 0:512],
                kt[0:64, k0 : k0 + 128],
                qt[0:64, q0 : q0 + 512],
                start=True,
                stop=True,
            )
            nc.tensor.matmul(
                st[:, 512:1024],
                kt[64:128, k0 : k0 + 128],
                qt[64:128, q0 : q0 + 512],
                start=True,
                stop=True,
            )
            pt = ptp.tile([128, 1024], BF16, tag="pt", name="pt")
            nc.scalar.activation(pt[:], st[:], EXP, scale=0.125)
            pts[(g, s)] = pt

        # QKV for batches 1-3 is a single stream drained 4 MMs/step (done by
        # step 36); the S^T+exp stream then runs AHEAD of the OT stream (up
        # to 2/step once the QKV stream is dry), so by groups 6-7 every exp
        # has already been computed and the tail is pure OT+cproj on the PE.
        qkv_stream = [(0, m) for m in range(32, 48)]
        qkv_stream += [(bn, m) for bn in (1, 2, 3) for m in range(48)]
        # batch 3's v fills are deferred into the exp-limited run-ahead phase
        # (steps 40+), where the PE would otherwise idle ~1us/step and trip
        # the HAM clock gate; they still finish well before group 6's OT
        late_stream = [(3, m) for m in range(32, 48)]
        qkv_stream = [u for u in qkv_stream if u not in late_stream]
        st_stream = [divmod(i, 8) for i in range(64)]
        sp = [0]

        def drain_qkv(n, gstep=0, late_ok=False):
            for _ in range(n):
                src_q = (
                    qkv_stream
                    if qkv_stream
                    else (late_stream if late_ok and gstep >= 40 else None)
                )
                if src_q:
                    r = emit_qkv_mm(*src_q.pop(0))
                    if r is not None:
                        pend_tp.append(r)

        def emit_sts(gstep):
            # mandatory: the current group's S^T is never later than its own
            # step; once the QKV stream is dry, run up to 2/step ahead
            # (capped so the pt pool never backs the ACT queue up on
            # not-yet-ready inputs)
            n = 1 if qkv_stream else 2
            for _ in range(n):
                if sp[0] < 64 and sp[0] <= gstep + 12:
                    emit_st_pair(*st_stream[sp[0]])
                    sp[0] += 1

        for g in range(8):
            b_, q2_ = divmod(g, 2)

            for s in range(8):
                gstep = g * 8 + s
                # --- softmax tail of group g-1 FIRST: its ACT copy and
                # DVE norm enter the queues ahead of the run-ahead exps ---
                if g >= 1:
                    if s == 0:
                        emit_bcast(g - 1, 0)
                    elif s == 1:
                        emit_norm(g - 1, 0)
                        emit_bcast(g - 1, 1)
                    elif s == 2:
                        emit_norm(g - 1, 1)
                    elif s == 3:
                        pc_, pt2_ = divmod(g - 1, 2)
                        cproj_fifo.extend((pc_, oc, pt2_) for oc in range(8))
                # --- OT pair: lag 4, but lag 2 in the last two groups
                # (their exps are pre-computed by the S^T run-ahead), which
                # pulls the final norm chains ~2 steps earlier ---
                lag = 2 if g >= 6 else 4
                if s >= lag:
                    emit_ot_pair(g, s - lag)
                drain_qkv(2, gstep, late_ok=True)
                # --- S^T stream (current group's step, or run-ahead) ---
                emit_sts(gstep)
                drain_qkv(2, gstep, late_ok=True)
                # pending V transposes (1 slot per step keeps PE dense)
                if pend_tp and s % 2 == 1:
                    emit_v_transposes(pend_tp.pop(0))
                # --- cproj from the FIFO, psum rotating through stP ---
                for ci in range(CPROJ_QUOTA[g]):
                    if cproj_fifo:
                        emit_cproj(
                            *cproj_fifo.pop(0), stP, "st", period=2 if g >= 6 else 4
                        )
                # bunched OT tail at the end of the group; filler between the
                # last pairs so OT(g,7) never waits on exp(g,7)
                if s == 7:
                    # the OT pairs themselves cover exp(g,7)'s latency; the
                    # recips go out BEFORE any other DVE work so the norm
                    # chain isn't queued behind copies
                    if lag == 4:
                        emit_ot_pair(g, 4)
                        emit_ot_pair(g, 5)
                    if cproj_fifo and CPROJ_QUOTA[g]:
                        emit_cproj(*cproj_fifo.pop(0), stP, "st", period=1)
                    emit_ot_pair(g, 6)
                    if cproj_fifo and CPROJ_QUOTA[g]:
                        emit_cproj(*cproj_fifo.pop(0), stP, "st", period=1)
                    emit_ot_pair(g, 7)
                    emit_recip(g, 0)
                    emit_recip(g, 1)
                    if pend_tp:
                        emit_v_transposes(pend_tp.pop(0))
                # copied cproj tiles -> HBM, spread across mid-group steps so
                # the sync queue is clear for the bc DMAs at boundaries
                if 1 <= s <= 6:
                    emit_out_dmas(4)

            while pend_tp:
                emit_v_transposes(pend_tp.pop(0))

        # ---------- epilogue ----------
        # Tail of group 7 interleaved with the remaining cproj matmuls
        # (leftovers only need norm(6), which is done; then g7's own 8).
        emit_bcast(7, 0)
        ep_pools = ((stP, "st"), (msP, "msc"), (qkvP, "fill"))
        ep_i = [0]

        def ep_cproj(n):
            for _ in range(n):
                if cproj_fifo:
                    pool, tag = ep_pools[ep_i[0] % 3]
                    ep_i[0] += 1
                    emit_cproj(*cproj_fifo.pop(0), pool, tag, period=2)

        ep_cproj(2)
        emit_norm(7, 0)
        emit_bcast(7, 1)
        ep_cproj(2)
        emit_norm(7, 1)
        EPQ = (nc.sync, nc.scalar)
        ep_cproj(len(cproj_fifo))
        emit_out_dmas(4, EPQ)
        cproj_fifo.extend((3, oc, 1) for oc in range(8))
        while cproj_fifo:
            ep_cproj(2)
            emit_out_dmas(2, EPQ)
        emit_out_dmas(len(cproj_done), EPQ)


def _build_nc():
    nc = bacc.Bacc(
        "TRN2",
        target_bir_lowering=False,
        debug=False,
        enable_asserts=False,
        num_devices=NCORES,
    )
    xtr = nc.dram_tensor("xtr", [128, 8, 8, 512], BF16, kind="ExternalInput").ap()
    wq = nc.dram_tensor("wq", [128, 1024], BF16, kind="ExternalInput").ap()
    wk = nc.dram_tensor("wk", [128, 1024], BF16, kind="ExternalInput").ap()
    wv = nc.dram_tensor("wv", [128, 1024], BF16, kind="ExternalInput").ap()
    wp = nc.dram_tensor("wp", [128, 1024], BF16, kind="ExternalInput").ap()
    bq = nc.dram_tensor("bq", [128, 1], F32, kind="ExternalInput").ap()
    bk = nc.dram_tensor("bk", [128, 1], F32, kind="ExternalInput").ap()
    bv = nc.dram_tensor("bv", [128, 1], F32, kind="ExternalInput").ap()
    identd = nc.dram_tensor("ident", [128, 128], BF16, kind="ExternalInput").ap()
    onecd = nc.dram_tensor("onec", [128, 64], BF16, kind="ExternalInput").ap()
    out = nc.dram_tensor("out_t", [NX, T], F16, kind="ExternalOutput").ap()
    with tile.TileContext(nc) as tc:
        _emit(nc, tc, xtr, wq, wk, wv, wp, bq, bk, bv, identd, onecd, out)
    nc.compile()
    return nc


def _pack_w(wcols):
    # [1024, 128] -> [128, 8*128] bf16: sbuf[p, k*128 + j] = W[k*128 + p, j]
    w = np.ascontiguousarray(np.asarray(wcols, dtype=np.float32))
    return np.ascontiguousarray(
        w.reshape(8, 128, 128).transpose(1, 0, 2).reshape(128, 1024)
    ).astype(ml_dtypes.bfloat16)


def _pack_xtr(X):
    # X [T, NX] -> xtr[p, qc, k, n] = X[qc*512+n, k*128+p]
    xt = np.asarray(X, dtype=np.float32).T  # [NX, T]
    xtr = xt.reshape(8, 128, 8, 512).transpose(1, 2, 0, 3)
    return np.ascontiguousarray(xtr).astype(ml_dtypes.bfloat16)


def kernel(hidden_states, c_attn_w, c_attn_b, c_proj_w, c_proj_b):
    global _nc_cache
    hidden_states = np.asarray(hidden_states, dtype=np.float32)
    c_attn_w = np.asarray(c_attn_w, dtype=np.float32)
    c_attn_b = np.asarray(c_attn_b, dtype=np.float32)
    c_proj_w = np.asarray(c_proj_w, dtype=np.float32)
    c_proj_b = np.asarray(c_proj_b, dtype=np.float32)

    if _nc_cache is None:
        _nc_cache = _build_nc()
    nc = _nc_cache

    X = hidden_states.reshape(T, NX)
    xtr_np = _pack_xtr(X)

    in_maps = []
    for c in range(NCORES):
        cs = slice(c * 128, (c + 1) * 128)
        in_maps.append(
            {
                "xtr": xtr_np,
                "wq": _pack_w(c_attn_w[:, c * 128 : (c + 1) * 128]),
                "wk": _pack_w(c_attn_w[:, 1024 + c * 128 : 1024 + (c + 1) * 128]),
                "wv": _pack_w(c_attn_w[:, 2048 + c * 128 : 2048 + (c + 1) * 128]),
                "wp": np.ascontiguousarray(c_proj_w[cs, :]).astype(ml_dtypes.bfloat16),
                "bq": np.ascontiguousarray(c_attn_b[cs].reshape(128, 1)),
                "bk": np.ascontiguousarray(
                    c_attn_b[1024 + c * 128 : 1024 + (c + 1) * 128].reshape(128, 1)
                ),
                "bv": np.ascontiguousarray(
                    c_attn_b[2048 + c * 128 : 2048 + (c + 1) * 128].reshape(128, 1)
                ),
                "ident": np.eye(128, dtype=np.float32).astype(ml_dtypes.bfloat16),
                "onec": np.ones((128, 64), dtype=ml_dtypes.bfloat16),
            }
        )

    trace = bool(int(os.environ.get("KERNEL_PROFILE", "0")))
    if trace:
        trace = _ensure_ntff_hook()
    try:
        res = run_bass_kernel_spmd(
            nc, in_maps, core_ids=list(range(NCORES)), trace=trace
        )
    except Exception:
        if not trace:
            raise
        print("kernel.py: traced run failed; retrying untraced")
        res = run_bass_kernel_spmd(nc, in_maps, core_ids=list(range(NCORES)))

    total = np.zeros((NX, T), np.float32)
    for r in res.results:
        total += r["out_t"].astype(np.float32)
    out = total.T.reshape(B, S, NX) + c_proj_b[None, None, :]
    kernel.last_exec_time_ns = res.exec_time_ns
    return out.astype(np.float32)
